# revision 48
# baseline (speedup 1.0000x reference)
"""Trainium2 Bass kernel for relative-position multi-head attention.

Math (per batch element b, head h):
    k = key @ Wk.T + bk, q = query @ Wq.T + bq, v = value @ Wv.T + bv
    R = pe @ Wr.T + br                       # [2L, HID]; rpe[i,j] = R[j-i+L]
    A + C = (q + u_bias) @ k.T               # u folded into q
    B + D = skew((q + v_bias) @ R_h.T)       # skew: [i, dd] -> [i, j], dd = j-i+L
    score = (A+B+C+D)/sqrt(DH), mask keys j >= seq_len, softmax over j
    out = (attn @ v) @ Wf.T + bf

Device design:
  - R is precomputed on the host (free) and shipped as fp8 (x64 scaled).
  - bf16 weights/activations on the main path; fp8e4m3 for the
    relative-position (B+D) path whose score contribution is tiny.
  - Compact skew: per 128-row i-tile only the 512-wide dd window
    [128*(2-it), +512) is computed/written.  The DRAM round trip (one
    write + one re-aligned read per HEAD PAIR) yields Bt[i, j] in [i, j]
    layout with NO XBAR transpose; the transpose to score layout [j, i]
    is done by fp8 matmuls against I/64 that accumulate directly into
    the scores PSUM, then A+C accumulates on top.
  - "scores transposed" layout [j (partitions), i (free)]: mask is a
    per-partition bias on the exp, denominator via a ones column in v.
  - Few large input DMAs; blob_q is split per-kt so the first projection
    starts as early as possible; PE warm-up matmuls ramp the clock.
  - One NeuronCore per batch element (data-parallel over batch).
"""

import sys

try:
    import concourse.bass as bass  # noqa: F401
except ImportError:
    sys.path.insert(0, "/opt/trn_rl_repo")

import ml_dtypes
import numpy as np

import concourse.bass as bass
import concourse.tile as tile
from concourse import bacc, mybir
from concourse.bass_utils import run_bass_kernel_spmd

F32 = mybir.dt.float32
BF16 = mybir.dt.bfloat16
FP8 = mybir.dt.float8e4
AF = mybir.ActivationFunctionType
OP = mybir.AluOpType

B, L, HID, NH, DH = 8, 384, 512, 8, 64
DD = 2 * L          # 768 distinct relative positions
NT = L // 128       # 3 token tiles
CT = HID // 128     # 4 channel tiles
W = 512             # compact skew window width per i-tile
QCH = HID + L       # 896: per-kt chunk [W_kt | x_kt] in the q/k/v blobs
SCALE = 1.0 / 8.0   # 1/sqrt(DH)
NEG = -30000.0      # mask bias; exp(x*SCALE + NEG) == 0.0 in fp32
RS = 64.0           # fp8 scale for the R / skew-scratch path

import os
N_WARM = int(os.environ.get("K_NWARM", "6"))
PST_BUFS = int(os.environ.get("K_PST", "4"))
F_INCR = os.environ.get("K_FINCR", "0") == "1"
READS_SP = os.environ.get("K_RSP", "1") == "1"
JUNK = os.environ.get("K_JUNK", "1") == "1"


def _build_program(skip_bias_rows: bool):
    nc = bacc.Bacc("TRN2", target_bir_lowering=False, debug=False, num_devices=8)

    def din(name, shape, dt=F32):
        return nc.dram_tensor(name, shape, dt, kind="ExternalInput").ap()

    # packed per-core inputs (channel-major activations, prepped on host)
    blob_q = din("blob_q", [128, CT * QCH], BF16)   # per kt: WqT_kt | qT_kt
    blob_k = din("blob_k", [128, CT * QCH], BF16)
    blob_v = din("blob_v", [128, CT * QCH], BF16)
    blob_f = din("blob_f", [128, CT * HID], BF16)   # WfT folded
    blob_r = din("blob_r", [128, 128 + CT * DD], FP8)  # I/64 | 64*R chan-major
    biases = din("biases", [128, CT * 4])   # folded cols [bq+u, bq+v, bk, 0]
    # misc row 0: [ones(128) | bv(512) | bf(512) | pad]; rows 0-1 cols 0:128: sel
    misc = din("misc", [2, 1280], BF16)
    seqlen = din("seqlen", [1, 1])

    out = nc.dram_tensor("out", [L, HID], BF16, kind="ExternalOutput").ap()
    skews = [nc.dram_tensor(f"skew{p}", [2 * NT * 128 * W + 512], FP8) for p in range(4)]

    with tile.TileContext(nc) as tc, nc.allow_low_precision(
        reason="bf16/fp8 mixed precision is intentional; accumulation is fp32 PSUM"
    ):
        _body(tc, locals())
    nc.compile()
    return nc


def _body(tc, io):
    nc = tc.nc
    skip_bias_rows = io["skip_bias_rows"]

    from contextlib import ExitStack

    with ExitStack() as ctx:
        consts = ctx.enter_context(tc.tile_pool(name="consts", bufs=1))
        work = ctx.enter_context(tc.tile_pool(name="work", bufs=1))
        s2_pool = ctx.enter_context(tc.tile_pool(name="s2b", bufs=4))
        bt_pool = ctx.enter_context(tc.tile_pool(name="bt", bufs=4))
        exp_pool = ctx.enter_context(tc.tile_pool(name="exps", bufs=1))
        osb_pool = ctx.enter_context(tc.tile_pool(name="osb", bufs=3))
        # PSUM is phased: psA (2 banks) global; psS2 (6) lives only through
        # the S2 phase, then its banks are reused by psT (4) + psV (2).
        psA = ctx.enter_context(tc.tile_pool(name="psA", bufs=2, space="PSUM"))
        s2_ctx = ExitStack()
        psS2 = s2_ctx.enter_context(tc.tile_pool(name="psS2", bufs=6, space="PSUM"))
        psT = psV = None  # opened after the S2 phase

        # ---- input DMAs (SP/HWDGE), deadline order; k/v after r, f last ----
        # Queue plan: SP = bias, q, k, w0, w1, v, w2, w3, f, outs (deadline
        # order); Pool = R, seqlen, misc, skew reads; Act = junk tails.
        # q/k/v chunks alternate between the SP and Act HWDGE queues so the
        # per-queue 650ns issue rate does not serialize the input stream.
        bq = consts.tile([128, CT, QCH], BF16, tag="bq", name="bq")
        _bqv = io["blob_q"].rearrange("p (a n) -> p a n", n=QCH)
        nc.sync.dma_start(out=bq[:, 0, :], in_=_bqv[:, 0, :])
        nc.scalar.dma_start(out=bq[:, 2, :], in_=_bqv[:, 2, :])
        bcols = consts.tile([128, CT * 4], F32, tag="biases", name="biases")
        nc.sync.dma_start(out=bcols, in_=io["biases"])
        nc.scalar.dma_start(out=bq[:, 3, :], in_=_bqv[:, 3, :])
        nc.sync.dma_start(out=bq[:, 1, :], in_=_bqv[:, 1, :])
        # R in two pair-halves on the Pool queue, generated after the seqlen
        # broadcast so their transfers trail the q chunks in the DMA FIFO.
        br_ = consts.tile([128, 128 + CT * DD], FP8, tag="br", name="br")

        ident_s = br_[:, 0:128]  # I/64 in fp8

        def wslice(blob, kt, ms):  # weight block [128, 128]
            return blob[:, kt, ms.start : ms.stop]

        def wfull(blob, kt):  # weight block [128, 512]
            return blob[:, kt, 0:HID]

        def xslice(blob, kt, sl):  # activation block [128, |sl|]
            return blob[:, kt, HID + sl.start : HID + sl.stop]

        def rslice(h, it):  # 64*R window for head h, i-tile it: [64, W]
            a, half = h // 2, (h % 2) * 64
            base = 128 + a * DD + 128 * (2 - it)
            return br_[half : half + 64, base : base + W]

        # ---- PE warm-up: ramp the clock while input DMAs are in flight.
        # Small memset so the first warm-up issues as early as possible.
        wtile = consts.tile([128, 128], BF16, tag="wtile", name="wtile")
        nc.vector.memset(wtile, 0.0)
        for w in range(4 * N_WARM):
            psw = psA.tile([128, 128], F32, tag="psA", name=f"warm{w}",
                           padded_shape=[128, 512])
            nc.tensor.matmul(psw, wtile, wtile, start=True, stop=True)

        # ---- seq_len -> per-partition additive mask columns (gpsimd) ----
        sl_bc = consts.tile([128, 1], F32, tag="sl_bc", name="sl_bc")
        nc.gpsimd.dma_start(
            out=sl_bc,
            in_=bass.AP(tensor=io["seqlen"].tensor, offset=0, ap=[[0, 128], [1, 1]]),
        )
        nc.gpsimd.dma_start(
            out=br_[:, 0 : 128 + 2 * DD], in_=io["blob_r"][:, 0 : 128 + 2 * DD]
        )
        nc.gpsimd.dma_start(
            out=br_[:, 128 + 2 * DD :], in_=io["blob_r"][:, 128 + 2 * DD :]
        )
        bk = consts.tile([128, CT, QCH], BF16, tag="bk", name="bk")
        _bkv = io["blob_k"].rearrange("p (a n) -> p a n", n=QCH)
        nc.sync.dma_start(out=bk[:, 0, :], in_=_bkv[:, 0, :])
        nc.scalar.dma_start(out=bk[:, 2, :], in_=_bkv[:, 2, :])
        nc.sync.dma_start(out=bk[:, 1, :], in_=_bkv[:, 1, :])
        nc.scalar.dma_start(out=bk[:, 3, :], in_=_bkv[:, 3, :])

        miscs = consts.tile([2, 1280], BF16, tag="misc", name="misc")
        nc.gpsimd.dma_start(out=miscs, in_=io["misc"])
        ones_row = miscs[0:1, 128:256]
        bv_row = miscs[0:1, 256:768]
        bf_row = miscs[0:1, 768:1280]

        if JUNK:
            # junk-tail writes: make the full-rate re-aligned reads below
            # end in initialized bytes (only needed under CoreSim checking).
            junk = consts.tile([1, 512], FP8, tag="junk", name="junk")
            nc.vector.memset(junk, 0.0)
            for p in range(4):
                nc.gpsimd.dma_start(
                    out=bass.AP(
                        tensor=io["skews"][p], offset=2 * NT * 128 * W,
                        ap=[[512, 1], [1, 512]],
                    ),
                    in_=junk,
                )
        masks = []
        for jt in range(NT):
            it_i32 = consts.tile([128, 1], mybir.dt.int32, tag=f"iota{jt}")
            nc.gpsimd.iota(it_i32, pattern=[[0, 1]], base=jt * 128, channel_multiplier=1)
            it_f32 = consts.tile([128, 1], F32, tag=f"iotaf{jt}")
            nc.vector.tensor_copy(out=it_f32, in_=it_i32)
            m01 = consts.tile([128, 1], F32, tag=f"m01_{jt}")
            nc.vector.tensor_tensor(out=m01, in0=it_f32, in1=sl_bc, op=OP.is_ge)
            mjt = consts.tile([128, 1], F32, tag=f"mask{jt}")
            nc.vector.tensor_scalar(
                out=mjt, in0=m01, scalar1=NEG, scalar2=None, op0=OP.mult
            )
            masks.append(mjt)

        qu_cm, qv_cm, k_cm = [None] * CT, [None] * CT, [None] * CT
        v_ext = [None] * NT
        ot_cm = [
            work.tile([128, L], BF16, tag=f"ot_cm{mt}", name=f"ot_cm{mt}")
            for mt in range(CT)
        ]
        s2_t = [None] * 4
        bt_t = [None] * 4
        exps_t = [None] * NH
        ppv_t = [None] * NH

        def proj_q(mt):
            ms = slice(mt * 128, (mt + 1) * 128)
            ps = psA.tile([128, 512], F32, tag="psA", name="psq")
            for kt in range(CT):
                nc.tensor.matmul(
                    ps[:, 0:L], wslice(bq, kt, ms), xslice(bq, kt, slice(0, L)).opt(),
                    start=(kt == 0), stop=(kt == CT - 1),
                )
            t = work.tile([128, L], FP8, tag=f"qv_cm{mt}", name=f"qv_cm{mt}")
            nc.vector.tensor_scalar(
                out=t, in0=ps[:, 0:L], scalar1=bcols[:, mt * 4 + 1 : mt * 4 + 2],
                scalar2=None, op0=OP.add,
            )
            qv_cm[mt] = t
            t = work.tile([128, L], BF16, tag=f"qu_cm{mt}", name=f"qu_cm{mt}")
            nc.scalar.activation(
                out=t, in_=ps[:, 0:L], func=AF.Identity,
                bias=bcols[:, mt * 4 : mt * 4 + 1],
            )
            qu_cm[mt] = t

        def proj_k(mt):
            ms = slice(mt * 128, (mt + 1) * 128)
            ps = psA.tile([128, 512], F32, tag="psA", name="psk")
            for kt in range(CT):
                nc.tensor.matmul(
                    ps[:, 0:L], wslice(bk, kt, ms), xslice(bk, kt, slice(0, L)).opt(),
                    start=(kt == 0), stop=(kt == CT - 1),
                )
            t = work.tile([128, L], BF16, tag=f"k_cm{mt}", name=f"k_cm{mt}")
            nc.vector.tensor_scalar(
                out=t, in0=ps[:, 0:L], scalar1=bcols[:, mt * 4 + 2 : mt * 4 + 3],
                scalar2=None, op0=OP.add,
            )
            k_cm[mt] = t

        def proj_v(it):
            # v token-major, packed per head: [64 v cols][1 ones][1 pad] x 8.
            # The ones column folds the softmax denominator into attn @ v.
            isl = slice(it * 128, (it + 1) * 128)
            ps = psA.tile([128, 512], F32, tag="psA", name="psv")
            for kt in range(CT):
                nc.tensor.matmul(
                    ps, xslice(bv_, kt, isl), wfull(bv_, kt).opt(),
                    start=(kt == 0), stop=(kt == CT - 1) and skip_bias_rows,
                )
            if not skip_bias_rows:
                nc.tensor.matmul(ps, ones_row, bv_row, start=False, stop=True)
            t = work.tile([128, NH, 66], BF16, tag=f"v_ext{it}", name=f"v_ext{it}")
            nc.vector.tensor_copy(
                out=t[:, :, 0:64], in_=ps.rearrange("p (h d) -> p h d", h=NH)
            )
            nc.vector.memset(t[:, :, 64:65], 1.0)
            v_ext[it] = t

        def s2_pair(p):
            """S2 = 64*(q + v_bias) @ R_h.T over compact windows for heads
            2p, 2p+1 into one SBUF staging tile (fp8)."""
            s2b = s2_pool.tile([128, 2, NT, W], FP8, tag="s2b", name="s2b")
            for hh in range(2):
                h = 2 * p + hh
                mt, half = h // 2, (h % 2) * 64
                hs = slice(half, half + 64)
                for it in range(NT):
                    isl = slice(it * 128, (it + 1) * 128)
                    ps2 = psS2.tile([128, W], F32, tag="s2", name="ps2")
                    nc.tensor.matmul(
                        ps2, qv_cm[mt][hs, isl], rslice(h, it),
                        start=True, stop=True, tile_position=(half, 0),
                    )
                    eng = (nc.vector, nc.scalar,
                           nc.vector if hh == 0 else nc.scalar)[it]
                    if eng is nc.scalar:
                        eng.copy(out=s2b[:, hh, it, :], in_=ps2)
                    else:
                        eng.tensor_copy(out=s2b[:, hh, it, :], in_=ps2)
            s2_t[p] = s2b

        def skew_write(p, hh=None):
            sk = io["skews"][p]
            rng = range(2) if hh is None else [hh]
            for h2 in rng:
                nc.sync.dma_start(
                    out=bass.AP(
                        tensor=sk, offset=h2 * NT * 128 * W,
                        ap=[[W, 128], [128 * W, NT], [1, W]],
                    ),
                    in_=s2_t[p][:, h2, :, :],
                )

        def skew_read(p):
            # re-aligned, per head half: bt[li, it, j] =
            #   scratch[hh*NT*128*W + it*128*W + li*(W-1) + j + 128]
            bts = []
            for hh in range(2):
                bt = bt_pool.tile([128, NT, W], FP8, tag=f"bt{hh}", name="bt")
                nc.sync.dma_start(
                    out=bt,
                    in_=bass.AP(
                        tensor=io["skews"][p], offset=hh * NT * 128 * W + 128,
                        ap=[[W - 1, 128], [128 * W, NT], [1, W]],
                    ),
                )
                bts.append(bt)
            bt_t[p] = bts

        def scores(h):
            """Scores: B+D via fp8 block-transpose matmuls against I/64 into
            PSUM, A+C accumulated on top; masked exp (scale folded); the
            attn @ v matmuls are interleaved so each fires as soon as its
            exp tile lands instead of waiting for all three."""
            mt, half = h // 2, (h % 2) * 64
            hs = slice(half, half + 64)
            bt = bt_t[h // 2][h % 2]
            exps = exp_pool.tile([128, NT, L], BF16, tag=f"exps{h % 3}", name="exps")
            for jt in range(NT):
                jsl = slice(jt * 128, (jt + 1) * 128)
                pst = psT.tile([128, L], F32, tag="pst", name="pst")
                for it in range(NT):
                    nc.tensor.matmul(
                        pst[:, it * 128 : (it + 1) * 128],
                        bt[:, it, jsl], ident_s,
                        start=True, stop=False,
                    )
                nc.tensor.matmul(
                    pst, k_cm[mt][hs, jsl], qu_cm[mt][hs, :],
                    start=False, stop=True, tile_position=(half, 0),
                )
                nc.scalar.activation(
                    out=exps[:, jt, :], in_=pst, func=AF.Exp,
                    bias=masks[jt], scale=SCALE,
                )
            exps_t[h] = exps

        def attn_v(h):
            # attn @ v; psum rows 0..63 = out_h.T, row 64 = sum_j exp
            ppv = psV.tile([65, L], F32, tag=f"ppv{h % 2}", name="ppv")
            for kt in range(NT):
                nc.tensor.matmul(
                    ppv, v_ext[kt][:, h, 0:65].opt(), exps_t[h][:, kt, :],
                    start=(kt == 0), stop=(kt == NT - 1),
                )
            ppv_t[h] = ppv
            rr = work.tile([1, L], BF16, tag=f"rr_{h % 4}", name="rr")
            nc.vector.reciprocal(out=rr, in_=ppv[64:65, :])
            rr_t[h] = rr

        rr_t = [None] * NH

        def norm_pair(mt):
            """Normalize heads 2mt, 2mt+1: per-head broadcast matmul of the
            reciprocal row, then a multiply into ot_cm."""
            h0, h1 = 2 * mt, 2 * mt + 1
            pbc0 = psT.tile([64, L], F32, tag="pst", name="pbc0")
            nc.tensor.matmul(
                pbc0, ones_row[0:1, 0:64], rr_t[h0], start=True, stop=True
            )
            rbc0 = work.tile([64, L], BF16, tag="rbc0", name="rbc0")
            nc.scalar.copy(out=rbc0, in_=pbc0)
            nc.vector.tensor_tensor(
                out=ot_cm[mt][0:64, :], in0=ppv_t[h0][0:64, :],
                in1=rbc0, op=OP.mult,
            )
            pbc1 = psT.tile([64, L], F32, tag="pst", name="pbc1")
            nc.tensor.matmul(
                pbc1, ones_row[0:1, 0:64], rr_t[h1], start=True, stop=True
            )
            rbc1 = work.tile([64, L], BF16, tag="rbc1", name="rbc1")
            nc.scalar.copy(out=rbc1, in_=pbc1)
            nc.vector.tensor_tensor(
                out=ot_cm[mt][64:128, :], in0=ppv_t[h1][0:64, :],
                in1=rbc1, op=OP.mult,
            )

        # ---- pipeline ----
        # q projections + S2 pairs first (they gate the DRAM round trips);
        # k/v projections fill PE while the round trips are in flight.
        proj_q(0)
        proj_q(1)
        s2_pair(0)
        skew_write(0)  # both halves (copies for both heads already queued)
        proj_q(2)
        s2_pair(1)
        skew_write(1)
        bv_ = consts.tile([128, CT, QCH], BF16, tag="bv", name="bv")
        _bvv = io["blob_v"].rearrange("p (a n) -> p a n", n=QCH)
        nc.sync.dma_start(out=bv_[:, 0, :], in_=_bvv[:, 0, :])
        nc.scalar.dma_start(out=bv_[:, 2, :], in_=_bvv[:, 2, :])
        nc.sync.dma_start(out=bv_[:, 1, :], in_=_bvv[:, 1, :])
        nc.scalar.dma_start(out=bv_[:, 3, :], in_=_bvv[:, 3, :])
        proj_q(3)
        s2_pair(2)
        skew_write(2)
        skew_read(0)
        s2_pair(3)
        skew_write(3)
        skew_read(1)
        skew_read(2)
        skew_read(3)

        # f blob load: dep-delayed (via the dummy copy below) so the
        # scheduler cannot hoist its transfer into the round-trip window.
        bf_ = consts.tile([128, CT, HID], BF16, tag="bf", name="bf")
        nc.vector.tensor_copy(out=bf_[0:1, 0, 0:1], in_=bt_t[3][1][0:1, 0, 0:1])
        nc.sync.dma_start(
            out=bf_, in_=io["blob_f"].rearrange("p (a n) -> p a n", n=HID)
        )

        def wfull_f(kt):
            return bf_[:, kt, 0:HID]

        proj_k(0)
        proj_k(1)
        for it in range(NT):
            proj_v(it)

        # phase 2: psA+psS2's banks are handed to the scores/final pools.
        s2_ctx.close()
        psT = ctx.enter_context(tc.tile_pool(name="psT", bufs=PST_BUFS, space="PSUM"))
        psV = ctx.enter_context(tc.tile_pool(name="psV", bufs=1, space="PSUM"))
        if F_INCR:
            psF = ctx.enter_context(tc.tile_pool(name="psF", bufs=1, space="PSUM"))
            psf_t = [
                psF.tile([128, 512], F32, tag=f"psf{it}", name=f"psf{it}")
                for it in range(NT)
            ]
        else:
            psF = psA

        def final_block(mt):
            # fold pair mt's feature block into the three output accumulators
            for it in range(NT):
                isl = slice(it * 128, (it + 1) * 128)
                nc.tensor.matmul(
                    psf_t[it], ot_cm[mt][:, isl], wfull_f(mt).opt(),
                    start=(mt == 0),
                    stop=(mt == CT - 1) and skip_bias_rows,
                )

        psf01 = [None, None, None]

        def f_partial(it):
            # accumulate kt0-2 early (ot_cm[0..2] long ready)
            isl = slice(it * 128, (it + 1) * 128)
            if it < 2:
                ps = psA.tile([128, 512], F32, tag="psA", name="psf")
            else:
                ps = psT.tile([128, 512], F32, tag="pst", name="psf2")
            for kt in range(NT):
                nc.tensor.matmul(
                    ps, ot_cm[kt][:, isl], wfull_f(kt).opt(),
                    start=(kt == 0), stop=False,
                )
            psf01[it] = ps

        def f_finish(it, ps, kts):
            isl = slice(it * 128, (it + 1) * 128)
            for kt in kts:
                nc.tensor.matmul(
                    ps, ot_cm[kt][:, isl], wfull_f(kt).opt(),
                    start=(kt == 0), stop=(kt == CT - 1) and skip_bias_rows,
                )
            if not skip_bias_rows:
                nc.tensor.matmul(ps, ones_row, bf_row, start=False, stop=True)
            osb = osb_pool.tile([128, 512], BF16, tag="osb", name="osb")
            if it == 1:
                nc.scalar.copy(out=osb, in_=ps)
            else:
                nc.vector.tensor_copy(out=osb, in_=ps)
            (nc.scalar if it == 2 else nc.sync).dma_start(
                out=io["out"][isl, :], in_=osb
            )

        for mt in range(CT):
            scores(2 * mt)
            scores(2 * mt + 1)
            attn_v(2 * mt)
            attn_v(2 * mt + 1)
            if mt < 2:
                proj_k(mt + 2)
            if mt == 3:
                f_partial(0)
                f_partial(1)
                f_partial(2)
            norm_pair(mt)
        for it in range(NT):
            f_finish(it, psf01[it], [CT - 1])


_CACHE = {}


def _get_nc(skip_bias_rows: bool):
    key = skip_bias_rows
    if key not in _CACHE:
        _CACHE[key] = _build_program(skip_bias_rows)
    return _CACHE[key]


def _fold(a):
    """[HID, N] -> [128, CT, N] channel-folded: row p, block a covers
    DRAM row a*128+p."""
    n = a.shape[1]
    return np.ascontiguousarray(a.reshape(CT, 128, n).transpose(1, 0, 2))


def prep_in_maps(inputs):
    """Host-side sharding + layout marshaling. Returns (in_maps, skip_bias_rows)."""
    f = np.float32
    bf = ml_dtypes.bfloat16
    f8 = ml_dtypes.float8_e4m3
    g = {k: np.asarray(v) for k, v in inputs.items()}

    # R = pe @ Wr.T + br computed on host; shipped channel-major, x64, fp8.
    R = (g["pe"].astype(f) @ g["Wr"].astype(f).T) + g["br"].astype(f)  # [DD, HID]
    r_fold = _fold(np.ascontiguousarray(R.T).astype(f) * RS).reshape(128, CT * DD)
    blob_r = np.concatenate(
        [np.eye(128, dtype=f) / RS, r_fold], axis=1
    ).astype(f8)

    wq = _fold(np.ascontiguousarray(g["Wq"].astype(f).T))  # [128, CT, HID]
    wk = _fold(np.ascontiguousarray(g["Wk"].astype(f).T))
    wv = _fold(np.ascontiguousarray(g["Wv"].astype(f).T))
    wf = _fold(np.ascontiguousarray(g["Wf"].astype(f).T))

    biases = np.stack(
        [
            g["bq"].astype(f) + g["u_bias"].astype(f).reshape(-1),
            g["bq"].astype(f) + g["v_bias"].astype(f).reshape(-1),
            g["bk"].astype(f),
            np.zeros(HID, f),
        ],
        axis=1,
    )  # [HID, 4] -> folded [128, CT*4]
    biases = np.ascontiguousarray(
        biases.reshape(CT, 128, 4).transpose(1, 0, 2).reshape(128, CT * 4)
    )

    misc = np.zeros((2, 1280), f)
    misc[0, 0:64] = 1.0     # sel row 0
    misc[1, 64:128] = 1.0   # sel row 1
    misc[0, 128:256] = 1.0  # ones row
    misc[0, 256:768] = g["bv"].astype(f)
    misc[0, 768:1280] = g["bf"].astype(f)
    misc = misc.astype(bf)

    skip_bias_rows = not (np.any(g["bv"]) or np.any(g["bf"]))

    shared = {
        "blob_r": blob_r,
        "biases": biases,
        "misc": misc,
        "blob_f": np.ascontiguousarray(wf.reshape(128, CT * HID)).astype(bf),
    }

    def qblob(wt, xt, dt=None):  # per-kt chunks [W_kt | x_kt] -> [128, CT*QCH]
        return np.ascontiguousarray(
            np.concatenate([wt, xt], axis=2).reshape(128, CT * QCH)
        ).astype(dt if dt is not None else bf)

    seq = np.asarray(g["seq_len"]).astype(np.int64)
    in_maps = []
    for b in range(B):
        m = dict(shared)
        qT = _fold(np.ascontiguousarray(g["query"][b].astype(f).T))
        kT = _fold(np.ascontiguousarray(g["key"][b].astype(f).T))
        vT = _fold(np.ascontiguousarray(g["value"][b].astype(f).T))
        m["blob_q"] = qblob(wq, qT)
        m["blob_k"] = qblob(wk, kT)
        m["blob_v"] = qblob(wv, vT)
        m["seqlen"] = np.array([[seq[b]]], dtype=f)
        in_maps.append(m)
    return in_maps, skip_bias_rows


def kernel(**inputs) -> np.ndarray:
    in_maps, skip_bias_rows = prep_in_maps(inputs)
    nc = _get_nc(skip_bias_rows)
    res = run_bass_kernel_spmd(nc, in_maps, list(range(B)))
    return np.stack([res.results[c]["out"] for c in range(B)]).astype(np.float32)


# revision 54
# speedup vs baseline: 1.0151x; 1.0151x over previous
"""Trainium2 Bass kernel for relative-position multi-head attention.

Math (per batch element b, head h):
    k = key @ Wk.T + bk, q = query @ Wq.T + bq, v = value @ Wv.T + bv
    R = pe @ Wr.T + br                       # [2L, HID]; rpe[i,j] = R[j-i+L]
    A + C = (q + u_bias) @ k.T               # u folded into q
    B + D = skew((q + v_bias) @ R_h.T)       # skew: [i, dd] -> [i, j], dd = j-i+L
    score = (A+B+C+D)/sqrt(DH), mask keys j >= seq_len, softmax over j
    out = (attn @ v) @ Wf.T + bf

Device design:
  - R is precomputed on the host (free) and shipped as fp8 (x64 scaled).
  - bf16 weights/activations on the main path; fp8e4m3 for the
    relative-position (B+D) path whose score contribution is tiny.
  - Compact skew: per 128-row i-tile only the 512-wide dd window
    [128*(2-it), +512) is computed/written.  The DRAM round trip (one
    write + one re-aligned read per HEAD PAIR) yields Bt[i, j] in [i, j]
    layout with NO XBAR transpose; the transpose to score layout [j, i]
    is done by fp8 matmuls against I/64 that accumulate directly into
    the scores PSUM, then A+C accumulates on top.
  - "scores transposed" layout [j (partitions), i (free)]: mask is a
    per-partition bias on the exp, denominator via a ones column in v.
  - Few large input DMAs; blob_q is split per-kt so the first projection
    starts as early as possible; PE warm-up matmuls ramp the clock.
  - One NeuronCore per batch element (data-parallel over batch).
"""

import sys

try:
    import concourse.bass as bass  # noqa: F401
except ImportError:
    sys.path.insert(0, "/opt/trn_rl_repo")

import ml_dtypes
import numpy as np

import concourse.bass as bass
import concourse.tile as tile
from concourse import bacc, mybir
from concourse.bass_utils import run_bass_kernel_spmd

F32 = mybir.dt.float32
BF16 = mybir.dt.bfloat16
FP8 = mybir.dt.float8e4
AF = mybir.ActivationFunctionType
OP = mybir.AluOpType

B, L, HID, NH, DH = 8, 384, 512, 8, 64
DD = 2 * L          # 768 distinct relative positions
NT = L // 128       # 3 token tiles
CT = HID // 128     # 4 channel tiles
W = 512             # compact skew window width per i-tile
QCH = HID + L       # 896: per-kt chunk [W_kt | x_kt] in the q/k/v blobs
SCALE = 1.0 / 8.0   # 1/sqrt(DH)
NEG = -30000.0      # mask bias; exp(x*SCALE + NEG) == 0.0 in fp32
RS = 64.0           # fp8 scale for the R / skew-scratch path

import os
N_WARM = int(os.environ.get("K_NWARM", "6"))
PST_BUFS = int(os.environ.get("K_PST", "4"))
F_INCR = os.environ.get("K_FINCR", "0") == "1"
READS_SP = os.environ.get("K_RSP", "1") == "1"
JUNK = os.environ.get("K_JUNK", "0") == "1"


def _build_program(skip_bias_rows: bool):
    nc = bacc.Bacc("TRN2", target_bir_lowering=False, debug=False, num_devices=8)

    def din(name, shape, dt=F32):
        return nc.dram_tensor(name, shape, dt, kind="ExternalInput").ap()

    # packed per-core inputs (channel-major activations, prepped on host)
    blob_q = din("blob_q", [128, CT * QCH], BF16)   # per kt: WqT_kt | qT_kt
    blob_k = din("blob_k", [128, CT * QCH], BF16)
    blob_v = din("blob_v", [128, CT * QCH], BF16)
    blob_f = din("blob_f", [128, CT * HID], BF16)   # WfT folded
    blob_r = din("blob_r", [128, 128 + CT * DD], FP8)  # I/64 | 64*R chan-major
    biases = din("biases", [128, CT * 4])   # folded cols [bq+u, bq+v, bk, 0]
    # misc row 0: [ones(128) | bv(512) | bf(512) | pad]; rows 0-1 cols 0:128: sel
    misc = din("misc", [2, 1280], BF16)
    seqlen = din("seqlen", [1, 1])

    out = nc.dram_tensor("out", [L, HID], BF16, kind="ExternalOutput").ap()
    skews = [nc.dram_tensor(f"skew{p}", [2 * NT * 128 * W + 512], FP8) for p in range(4)]

    with tile.TileContext(nc) as tc, nc.allow_low_precision(
        reason="bf16/fp8 mixed precision is intentional; accumulation is fp32 PSUM"
    ):
        _body(tc, locals())
    nc.compile()
    return nc


def _body(tc, io):
    nc = tc.nc
    skip_bias_rows = io["skip_bias_rows"]

    from contextlib import ExitStack

    with ExitStack() as ctx:
        consts = ctx.enter_context(tc.tile_pool(name="consts", bufs=1))
        work = ctx.enter_context(tc.tile_pool(name="work", bufs=1))
        s2_pool = ctx.enter_context(tc.tile_pool(name="s2b", bufs=4))
        bt_pool = ctx.enter_context(tc.tile_pool(name="bt", bufs=4))
        exp_pool = ctx.enter_context(tc.tile_pool(name="exps", bufs=1))
        osb_pool = ctx.enter_context(tc.tile_pool(name="osb", bufs=3))
        # PSUM is phased: psA (2 banks) global; psS2 (6) lives only through
        # the S2 phase, then its banks are reused by psT (4) + psV (2).
        psA = ctx.enter_context(tc.tile_pool(name="psA", bufs=2, space="PSUM"))
        s2_ctx = ExitStack()
        psS2 = s2_ctx.enter_context(tc.tile_pool(name="psS2", bufs=6, space="PSUM"))
        psT = psV = None  # opened after the S2 phase

        # ---- input DMAs (SP/HWDGE), deadline order; k/v after r, f last ----
        # Queue plan: SP = bias, q, k, w0, w1, v, w2, w3, f, outs (deadline
        # order); Pool = R, seqlen, misc, skew reads; Act = junk tails.
        # q/k/v chunks alternate between the SP and Act HWDGE queues so the
        # per-queue 650ns issue rate does not serialize the input stream.
        bq = consts.tile([128, CT, QCH], BF16, tag="bq", name="bq")
        _bqv = io["blob_q"].rearrange("p (a n) -> p a n", n=QCH)
        nc.sync.dma_start(out=bq[:, 0, :], in_=_bqv[:, 0, :])
        nc.scalar.dma_start(out=bq[:, 2, :], in_=_bqv[:, 2, :])
        bcols = consts.tile([128, CT * 4], F32, tag="biases", name="biases")
        nc.sync.dma_start(out=bcols, in_=io["biases"])
        nc.scalar.dma_start(out=bq[:, 3, :], in_=_bqv[:, 3, :])
        nc.sync.dma_start(out=bq[:, 1, :], in_=_bqv[:, 1, :])
        # R in two pair-halves on the Pool queue, generated after the seqlen
        # broadcast so their transfers trail the q chunks in the DMA FIFO.
        br_ = consts.tile([128, 128 + CT * DD], FP8, tag="br", name="br")

        ident_s = br_[:, 0:128]  # I/64 in fp8

        def wslice(blob, kt, ms):  # weight block [128, 128]
            return blob[:, kt, ms.start : ms.stop]

        def wfull(blob, kt):  # weight block [128, 512]
            return blob[:, kt, 0:HID]

        def xslice(blob, kt, sl):  # activation block [128, |sl|]
            return blob[:, kt, HID + sl.start : HID + sl.stop]

        def rslice(h, it):  # 64*R window for head h, i-tile it: [64, W]
            a, half = h // 2, (h % 2) * 64
            base = 128 + a * DD + 128 * (2 - it)
            return br_[half : half + 64, base : base + W]

        # ---- PE warm-up: ramp the clock while input DMAs are in flight.
        # Small memset so the first warm-up issues as early as possible.
        wtile = consts.tile([128, 128], BF16, tag="wtile", name="wtile")
        nc.vector.memset(wtile, 0.0)
        for w in range(4 * N_WARM):
            psw = psA.tile([128, 128], F32, tag="psA", name=f"warm{w}",
                           padded_shape=[128, 512])
            nc.tensor.matmul(psw, wtile, wtile, start=True, stop=True)

        # ---- seq_len -> per-partition additive mask columns (gpsimd) ----
        sl_bc = consts.tile([128, 1], F32, tag="sl_bc", name="sl_bc")
        nc.gpsimd.dma_start(
            out=sl_bc,
            in_=bass.AP(tensor=io["seqlen"].tensor, offset=0, ap=[[0, 128], [1, 1]]),
        )
        nc.gpsimd.dma_start(
            out=br_[:, 0 : 128 + 2 * DD], in_=io["blob_r"][:, 0 : 128 + 2 * DD]
        )
        nc.gpsimd.dma_start(
            out=br_[:, 128 + 2 * DD :], in_=io["blob_r"][:, 128 + 2 * DD :]
        )
        bk = consts.tile([128, CT, QCH], BF16, tag="bk", name="bk")
        _bkv = io["blob_k"].rearrange("p (a n) -> p a n", n=QCH)
        nc.sync.dma_start(out=bk[:, 0, :], in_=_bkv[:, 0, :])
        nc.scalar.dma_start(out=bk[:, 2, :], in_=_bkv[:, 2, :])
        nc.sync.dma_start(out=bk[:, 1, :], in_=_bkv[:, 1, :])
        nc.scalar.dma_start(out=bk[:, 3, :], in_=_bkv[:, 3, :])

        miscs = consts.tile([2, 1280], BF16, tag="misc", name="misc")
        nc.gpsimd.dma_start(out=miscs, in_=io["misc"])
        ones_row = miscs[0:1, 128:256]
        bv_row = miscs[0:1, 256:768]
        bf_row = miscs[0:1, 768:1280]

        if JUNK:
            # junk-tail writes: make the full-rate re-aligned reads below
            # end in initialized bytes (only needed under CoreSim checking).
            junk = consts.tile([1, 512], FP8, tag="junk", name="junk")
            nc.vector.memset(junk, 0.0)
            for p in range(4):
                nc.gpsimd.dma_start(
                    out=bass.AP(
                        tensor=io["skews"][p], offset=2 * NT * 128 * W,
                        ap=[[512, 1], [1, 512]],
                    ),
                    in_=junk,
                )
        masks = []
        for jt in range(NT):
            it_i32 = consts.tile([128, 1], mybir.dt.int32, tag=f"iota{jt}")
            nc.gpsimd.iota(it_i32, pattern=[[0, 1]], base=jt * 128, channel_multiplier=1)
            it_f32 = consts.tile([128, 1], F32, tag=f"iotaf{jt}")
            nc.vector.tensor_copy(out=it_f32, in_=it_i32)
            m01 = consts.tile([128, 1], F32, tag=f"m01_{jt}")
            nc.vector.tensor_tensor(out=m01, in0=it_f32, in1=sl_bc, op=OP.is_ge)
            mjt = consts.tile([128, 1], F32, tag=f"mask{jt}")
            nc.vector.tensor_scalar(
                out=mjt, in0=m01, scalar1=NEG, scalar2=None, op0=OP.mult
            )
            masks.append(mjt)

        qu_cm, qv_cm, k_cm = [None] * CT, [None] * CT, [None] * CT
        v_ext = [None] * NT
        ot_cm = [
            work.tile([128, L], BF16, tag=f"ot_cm{mt}", name=f"ot_cm{mt}")
            for mt in range(CT)
        ]
        s2_t = [None] * 4
        bt_t = [None] * 4
        exps_t = [None] * NH
        ppv_t = [None] * NH

        def proj_q(mt):
            ms = slice(mt * 128, (mt + 1) * 128)
            ps = psA.tile([128, 512], F32, tag="psA", name="psq")
            for kt in range(CT):
                nc.tensor.matmul(
                    ps[:, 0:L], wslice(bq, kt, ms), xslice(bq, kt, slice(0, L)).opt(),
                    start=(kt == 0), stop=(kt == CT - 1),
                )
            t = work.tile([128, L], FP8, tag=f"qv_cm{mt}", name=f"qv_cm{mt}")
            nc.vector.tensor_scalar(
                out=t, in0=ps[:, 0:L], scalar1=bcols[:, mt * 4 + 1 : mt * 4 + 2],
                scalar2=None, op0=OP.add,
            )
            qv_cm[mt] = t
            t = work.tile([128, L], BF16, tag=f"qu_cm{mt}", name=f"qu_cm{mt}")
            nc.scalar.activation(
                out=t, in_=ps[:, 0:L], func=AF.Identity,
                bias=bcols[:, mt * 4 : mt * 4 + 1],
            )
            qu_cm[mt] = t

        def proj_k(mt):
            ms = slice(mt * 128, (mt + 1) * 128)
            ps = psA.tile([128, 512], F32, tag="psA", name="psk")
            for kt in range(CT):
                nc.tensor.matmul(
                    ps[:, 0:L], wslice(bk, kt, ms), xslice(bk, kt, slice(0, L)).opt(),
                    start=(kt == 0), stop=(kt == CT - 1),
                )
            t = work.tile([128, L], BF16, tag=f"k_cm{mt}", name=f"k_cm{mt}")
            nc.vector.tensor_scalar(
                out=t, in0=ps[:, 0:L], scalar1=bcols[:, mt * 4 + 2 : mt * 4 + 3],
                scalar2=None, op0=OP.add,
            )
            k_cm[mt] = t

        def proj_v(it):
            # v token-major, packed per head: [64 v cols][1 ones][1 pad] x 8.
            # The ones column folds the softmax denominator into attn @ v.
            isl = slice(it * 128, (it + 1) * 128)
            ps = psA.tile([128, 512], F32, tag="psA", name="psv")
            for kt in range(CT):
                nc.tensor.matmul(
                    ps, xslice(bv_, kt, isl), wfull(bv_, kt).opt(),
                    start=(kt == 0), stop=(kt == CT - 1) and skip_bias_rows,
                )
            if not skip_bias_rows:
                nc.tensor.matmul(ps, ones_row, bv_row, start=False, stop=True)
            t = work.tile([128, NH, 66], BF16, tag=f"v_ext{it}", name=f"v_ext{it}")
            nc.vector.tensor_copy(
                out=t[:, :, 0:64], in_=ps.rearrange("p (h d) -> p h d", h=NH)
            )
            nc.vector.memset(t[:, :, 64:65], 1.0)
            v_ext[it] = t

        def s2_pair(p):
            """S2 = 64*(q + v_bias) @ R_h.T over compact windows for heads
            2p, 2p+1 into one SBUF staging tile (fp8)."""
            s2b = s2_pool.tile([128, 2, NT, W], FP8, tag="s2b", name="s2b")
            for hh in range(2):
                h = 2 * p + hh
                mt, half = h // 2, (h % 2) * 64
                hs = slice(half, half + 64)
                for it in range(NT):
                    isl = slice(it * 128, (it + 1) * 128)
                    ps2 = psS2.tile([128, W], F32, tag="s2", name="ps2")
                    nc.tensor.matmul(
                        ps2, qv_cm[mt][hs, isl], rslice(h, it),
                        start=True, stop=True, tile_position=(half, 0),
                    )
                    eng = (nc.vector, nc.scalar,
                           nc.vector if hh == 0 else nc.scalar)[it]
                    if eng is nc.scalar:
                        eng.copy(out=s2b[:, hh, it, :], in_=ps2)
                    else:
                        eng.tensor_copy(out=s2b[:, hh, it, :], in_=ps2)
            s2_t[p] = s2b

        def skew_write(p, hh=None):
            sk = io["skews"][p]
            rng = range(2) if hh is None else [hh]
            for h2 in rng:
                nc.sync.dma_start(
                    out=bass.AP(
                        tensor=sk, offset=h2 * NT * 128 * W,
                        ap=[[W, 128], [128 * W, NT], [1, W]],
                    ),
                    in_=s2_t[p][:, h2, :, :],
                )

        def skew_read(p):
            # re-aligned, per head half: bt[li, it, j] =
            #   scratch[hh*NT*128*W + it*128*W + li*(W-1) + j + 128]
            bts = []
            for hh in range(2):
                bt = bt_pool.tile([128, NT, W], FP8, tag=f"bt{hh}", name="bt")
                nc.sync.dma_start(
                    out=bt,
                    in_=bass.AP(
                        tensor=io["skews"][p], offset=hh * NT * 128 * W + 128,
                        ap=[[W - 1, 128], [128 * W, NT], [1, W]],
                    ),
                )
                bts.append(bt)
            bt_t[p] = bts

        def scores(h):
            """Scores: B+D via fp8 block-transpose matmuls against I/64 into
            PSUM, A+C accumulated on top; masked exp (scale folded); the
            attn @ v matmuls are interleaved so each fires as soon as its
            exp tile lands instead of waiting for all three."""
            mt, half = h // 2, (h % 2) * 64
            hs = slice(half, half + 64)
            bt = bt_t[h // 2][h % 2]
            exps = exp_pool.tile([128, NT, L], BF16, tag=f"exps{h % 3}", name="exps")
            for jt in range(NT):
                jsl = slice(jt * 128, (jt + 1) * 128)
                pst = psT.tile([128, L], F32, tag="pst", name="pst")
                for it in range(NT):
                    nc.tensor.matmul(
                        pst[:, it * 128 : (it + 1) * 128],
                        bt[:, it, jsl], ident_s,
                        start=True, stop=False,
                    )
                nc.tensor.matmul(
                    pst, k_cm[mt][hs, jsl], qu_cm[mt][hs, :],
                    start=False, stop=True, tile_position=(half, 0),
                )
                nc.scalar.activation(
                    out=exps[:, jt, :], in_=pst, func=AF.Exp,
                    bias=masks[jt], scale=SCALE,
                )
            exps_t[h] = exps

        def attn_v(h):
            # attn @ v; psum rows 0..63 = out_h.T, row 64 = sum_j exp
            ppv = psV.tile([65, L], F32, tag=f"ppv{h % 2}", name="ppv")
            for kt in range(NT):
                nc.tensor.matmul(
                    ppv, v_ext[kt][:, h, 0:65].opt(), exps_t[h][:, kt, :],
                    start=(kt == 0), stop=(kt == NT - 1),
                )
            ppv_t[h] = ppv
            rr = work.tile([1, L], BF16, tag=f"rr_{h % 4}", name="rr")
            nc.vector.reciprocal(out=rr, in_=ppv[64:65, :])
            rr_t[h] = rr

        rr_t = [None] * NH

        def norm_pair(mt):
            """Normalize heads 2mt, 2mt+1: per-head broadcast matmul of the
            reciprocal row, then a multiply into ot_cm."""
            h0, h1 = 2 * mt, 2 * mt + 1
            pbc0 = psT.tile([64, L], F32, tag="pst", name="pbc0")
            nc.tensor.matmul(
                pbc0, ones_row[0:1, 0:64], rr_t[h0], start=True, stop=True
            )
            rbc0 = work.tile([64, L], BF16, tag="rbc0", name="rbc0")
            nc.scalar.copy(out=rbc0, in_=pbc0)
            nc.vector.tensor_tensor(
                out=ot_cm[mt][0:64, :], in0=ppv_t[h0][0:64, :],
                in1=rbc0, op=OP.mult,
            )
            pbc1 = psT.tile([64, L], F32, tag="pst", name="pbc1")
            nc.tensor.matmul(
                pbc1, ones_row[0:1, 0:64], rr_t[h1], start=True, stop=True
            )
            rbc1 = work.tile([64, L], BF16, tag="rbc1", name="rbc1")
            nc.scalar.copy(out=rbc1, in_=pbc1)
            nc.vector.tensor_tensor(
                out=ot_cm[mt][64:128, :], in0=ppv_t[h1][0:64, :],
                in1=rbc1, op=OP.mult,
            )

        # ---- pipeline ----
        # q projections + S2 pairs first (they gate the DRAM round trips);
        # k/v projections fill PE while the round trips are in flight.
        proj_q(0)
        proj_q(1)
        s2_pair(0)
        skew_write(0)  # both halves (copies for both heads already queued)
        proj_q(2)
        s2_pair(1)
        skew_write(1)
        bv_ = consts.tile([128, CT, QCH], BF16, tag="bv", name="bv")
        _bvv = io["blob_v"].rearrange("p (a n) -> p a n", n=QCH)
        nc.sync.dma_start(out=bv_[:, 0, :], in_=_bvv[:, 0, :])
        nc.scalar.dma_start(out=bv_[:, 2, :], in_=_bvv[:, 2, :])
        nc.sync.dma_start(out=bv_[:, 1, :], in_=_bvv[:, 1, :])
        nc.scalar.dma_start(out=bv_[:, 3, :], in_=_bvv[:, 3, :])
        proj_q(3)
        s2_pair(2)
        skew_write(2)
        skew_read(0)
        s2_pair(3)
        skew_write(3)
        skew_read(1)
        skew_read(2)
        skew_read(3)

        # f blob load: dep-delayed (via the dummy copy below) so the
        # scheduler cannot hoist its transfer into the round-trip window.
        bf_ = consts.tile([128, CT, HID], BF16, tag="bf", name="bf")
        nc.vector.tensor_copy(out=bf_[0:1, 0, 0:1], in_=bt_t[3][1][0:1, 0, 0:1])
        nc.sync.dma_start(
            out=bf_, in_=io["blob_f"].rearrange("p (a n) -> p a n", n=HID)
        )

        def wfull_f(kt):
            return bf_[:, kt, 0:HID]

        proj_k(0)
        proj_k(1)
        for it in range(NT):
            proj_v(it)

        # phase 2: psA+psS2's banks are handed to the scores/final pools.
        s2_ctx.close()
        psT = ctx.enter_context(tc.tile_pool(name="psT", bufs=PST_BUFS, space="PSUM"))
        psV = ctx.enter_context(tc.tile_pool(name="psV", bufs=1, space="PSUM"))
        if F_INCR:
            psF = ctx.enter_context(tc.tile_pool(name="psF", bufs=1, space="PSUM"))
            psf_t = [
                psF.tile([128, 512], F32, tag=f"psf{it}", name=f"psf{it}")
                for it in range(NT)
            ]
        else:
            psF = psA

        def final_block(mt):
            # fold pair mt's feature block into the three output accumulators
            for it in range(NT):
                isl = slice(it * 128, (it + 1) * 128)
                nc.tensor.matmul(
                    psf_t[it], ot_cm[mt][:, isl], wfull_f(mt).opt(),
                    start=(mt == 0),
                    stop=(mt == CT - 1) and skip_bias_rows,
                )

        psf01 = [None, None, None]

        def f_partial(it):
            # accumulate kt0-2 early (ot_cm[0..2] long ready)
            isl = slice(it * 128, (it + 1) * 128)
            if it < 2:
                ps = psA.tile([128, 512], F32, tag="psA", name="psf")
            else:
                ps = psT.tile([128, 512], F32, tag="pst", name="psf2")
            for kt in range(NT):
                nc.tensor.matmul(
                    ps, ot_cm[kt][:, isl], wfull_f(kt).opt(),
                    start=(kt == 0), stop=False,
                )
            psf01[it] = ps

        def f_finish(it, ps, kts):
            isl = slice(it * 128, (it + 1) * 128)
            for kt in kts:
                nc.tensor.matmul(
                    ps, ot_cm[kt][:, isl], wfull_f(kt).opt(),
                    start=(kt == 0), stop=(kt == CT - 1) and skip_bias_rows,
                )
            if not skip_bias_rows:
                nc.tensor.matmul(ps, ones_row, bf_row, start=False, stop=True)
            osb = osb_pool.tile([128, 512], BF16, tag="osb", name="osb")
            if it == 1:
                nc.scalar.copy(out=osb, in_=ps)
            else:
                nc.vector.tensor_copy(out=osb, in_=ps)
            (nc.scalar if it == 2 else nc.sync).dma_start(
                out=io["out"][isl, :], in_=osb
            )

        for mt in range(CT):
            scores(2 * mt)
            scores(2 * mt + 1)
            attn_v(2 * mt)
            attn_v(2 * mt + 1)
            if mt < 2:
                proj_k(mt + 2)
            if mt == 3:
                f_partial(0)
                f_partial(1)
                f_partial(2)
            norm_pair(mt)
        for it in range(NT):
            f_finish(it, psf01[it], [CT - 1])


_CACHE = {}


def _get_nc(skip_bias_rows: bool):
    key = skip_bias_rows
    if key not in _CACHE:
        _CACHE[key] = _build_program(skip_bias_rows)
    return _CACHE[key]


def _fold(a):
    """[HID, N] -> [128, CT, N] channel-folded: row p, block a covers
    DRAM row a*128+p."""
    n = a.shape[1]
    return np.ascontiguousarray(a.reshape(CT, 128, n).transpose(1, 0, 2))


def prep_in_maps(inputs):
    """Host-side sharding + layout marshaling. Returns (in_maps, skip_bias_rows)."""
    f = np.float32
    bf = ml_dtypes.bfloat16
    f8 = ml_dtypes.float8_e4m3
    g = {k: np.asarray(v) for k, v in inputs.items()}

    # R = pe @ Wr.T + br computed on host; shipped channel-major, x64, fp8.
    R = (g["pe"].astype(f) @ g["Wr"].astype(f).T) + g["br"].astype(f)  # [DD, HID]
    r_fold = _fold(np.ascontiguousarray(R.T).astype(f) * RS).reshape(128, CT * DD)
    blob_r = np.concatenate(
        [np.eye(128, dtype=f) / RS, r_fold], axis=1
    ).astype(f8)

    wq = _fold(np.ascontiguousarray(g["Wq"].astype(f).T))  # [128, CT, HID]
    wk = _fold(np.ascontiguousarray(g["Wk"].astype(f).T))
    wv = _fold(np.ascontiguousarray(g["Wv"].astype(f).T))
    wf = _fold(np.ascontiguousarray(g["Wf"].astype(f).T))

    biases = np.stack(
        [
            g["bq"].astype(f) + g["u_bias"].astype(f).reshape(-1),
            g["bq"].astype(f) + g["v_bias"].astype(f).reshape(-1),
            g["bk"].astype(f),
            np.zeros(HID, f),
        ],
        axis=1,
    )  # [HID, 4] -> folded [128, CT*4]
    biases = np.ascontiguousarray(
        biases.reshape(CT, 128, 4).transpose(1, 0, 2).reshape(128, CT * 4)
    )

    misc = np.zeros((2, 1280), f)
    misc[0, 0:64] = 1.0     # sel row 0
    misc[1, 64:128] = 1.0   # sel row 1
    misc[0, 128:256] = 1.0  # ones row
    misc[0, 256:768] = g["bv"].astype(f)
    misc[0, 768:1280] = g["bf"].astype(f)
    misc = misc.astype(bf)

    skip_bias_rows = not (np.any(g["bv"]) or np.any(g["bf"]))

    shared = {
        "blob_r": blob_r,
        "biases": biases,
        "misc": misc,
        "blob_f": np.ascontiguousarray(wf.reshape(128, CT * HID)).astype(bf),
    }

    def qblob(wt, xt, dt=None):  # per-kt chunks [W_kt | x_kt] -> [128, CT*QCH]
        return np.ascontiguousarray(
            np.concatenate([wt, xt], axis=2).reshape(128, CT * QCH)
        ).astype(dt if dt is not None else bf)

    seq = np.asarray(g["seq_len"]).astype(np.int64)
    in_maps = []
    for b in range(B):
        m = dict(shared)
        qT = _fold(np.ascontiguousarray(g["query"][b].astype(f).T))
        kT = _fold(np.ascontiguousarray(g["key"][b].astype(f).T))
        vT = _fold(np.ascontiguousarray(g["value"][b].astype(f).T))
        m["blob_q"] = qblob(wq, qT)
        m["blob_k"] = qblob(wk, kT)
        m["blob_v"] = qblob(wv, vT)
        m["seqlen"] = np.array([[seq[b]]], dtype=f)
        in_maps.append(m)
    return in_maps, skip_bias_rows


def kernel(**inputs) -> np.ndarray:
    in_maps, skip_bias_rows = prep_in_maps(inputs)
    nc = _get_nc(skip_bias_rows)
    res = run_bass_kernel_spmd(nc, in_maps, list(range(B)))
    return np.stack([res.results[c]["out"] for c in range(B)]).astype(np.float32)
